# revision 5
# baseline (speedup 1.0000x reference)
"""Trainium2 Bass kernel for nn_Attention_91225105367483.

Spatial attention: x[B=2,T=8,H=32,W=32,D=768] -> 16 frames x 1024 tokens.
Data-parallel over frames: 8 cores x 2 frames each. No collectives.

Per-core layout strategy (all hardcoded):
  - host pre-transposes: xT [768, 2048] (d-major), wqkvT [768, 2304],
    w_outT [768, 768], RoPE cos/sin packed [128, SL] (2 heads x 64 dims),
    rotation matrix RT [128,128] implementing rotate_half as a matmul.
  - QKV proj: q,k computed TRANSPOSED ([64 hd, seq]) so attention needs no
    on-chip transposes; v computed natural ([seq, 64]) with a ones column
    appended per head so attn@v also emits softmax denominators.
  - scores computed transposed sT=[keys, queries]; exp on ACT engine with
    scale=1/8 folded in, output bf16 straight to SBUF.
  - softmax skips max-subtraction (scores are O(1); exp is exact to 2ULP
    and fp32 accumulation makes this identical to the reference softmax).
  - o accumulated as [65, 512] in PSUM (row 64 = denominator); normalize via
    DVE reciprocal + PE broadcast matmul; out-proj from transposed o gives
    natural-layout output tiles.
"""
import sys

sys.path.insert(0, "/opt/trn_rl_repo")

import numpy as np
import ml_dtypes

BF16 = ml_dtypes.bfloat16

B, T, D = 2, 8, 768
NH, HD = 12, 64
NCORES = 8
FPC = 2  # frames per core

_built = {}


def _host_rope(H, W, head_dim):
    """Replicates reference._rope_cos_sin in numpy fp32."""
    half = head_dim // 4
    inv_freq = (1.0 / (10000.0 ** (np.arange(half, dtype=np.float32) / half))).astype(
        np.float32
    )
    th_h = np.arange(H, dtype=np.float32)[:, None] * inv_freq  # [H, half]
    th_w = np.arange(W, dtype=np.float32)[:, None] * inv_freq  # [W, half]
    cos = np.concatenate(
        [
            np.broadcast_to(np.cos(th_h)[:, None, :], (H, W, half)),
            np.broadcast_to(np.cos(th_w)[None, :, :], (H, W, half)),
        ],
        axis=-1,
    )
    sin = np.concatenate(
        [
            np.broadcast_to(np.sin(th_h)[:, None, :], (H, W, half)),
            np.broadcast_to(np.sin(th_w)[None, :, :], (H, W, half)),
        ],
        axis=-1,
    )
    cos = np.repeat(cos, 2, axis=-1).reshape(H * W, head_dim).astype(np.float32)
    sin = np.repeat(sin, 2, axis=-1).reshape(H * W, head_dim).astype(np.float32)
    return cos, sin


def _rot_matT():
    """RT = R.T where (R @ q)[2i] = -q[2i+1], (R @ q)[2i+1] = q[2i]."""
    RT = np.zeros((128, 128), dtype=np.float32)
    for i in range(64):
        RT[2 * i + 1, 2 * i] = -1.0
        RT[2 * i, 2 * i + 1] = 1.0
    return RT


def build_nc(H, W):
    """Builds the per-core Bass program. S = H*W tokens per frame."""
    import concourse.bass as bass
    import concourse.tile as tile
    from concourse import bacc, mybir

    dt = mybir.dt
    S = H * W
    SL = FPC * S  # tokens per core
    QCS = min(512, S)  # query-chunk size
    NQC = S // QCS  # query chunks per frame
    KT = S // 128  # key tiles per frame
    GK = min(2, KT)  # key tiles per exp group PER HEAD (2 heads share the tile)
    NG = KT // GK  # groups
    SC = min(512, SL)  # token chunk for qkv proj
    NSC = SL // SC
    NST = SL // 128  # s-tiles per core
    NDC = D // 128  # 6 d-chunks
    NET = 3 * D // 128  # 18 e-tiles of qkv output
    NPAIR = NH // 2  # 6 head pairs

    nc = bacc.Bacc("TRN2", target_bir_lowering=False, debug=False)

    xT = nc.dram_tensor("xT", [D, SL], dt.bfloat16, kind="ExternalInput")
    wqkvT = nc.dram_tensor("wqkvT", [D, 3 * D], dt.bfloat16, kind="ExternalInput")
    w_outT = nc.dram_tensor("w_outT", [D, D], dt.bfloat16, kind="ExternalInput")
    cosP = nc.dram_tensor("cosP", [128, SL], dt.bfloat16, kind="ExternalInput")
    sinP = nc.dram_tensor("sinP", [128, SL], dt.bfloat16, kind="ExternalInput")
    rotT = nc.dram_tensor("rotT", [128, 128], dt.bfloat16, kind="ExternalInput")
    ones64 = nc.dram_tensor("ones64", [1, 64], dt.bfloat16, kind="ExternalInput")
    bias_rep = nc.dram_tensor("bias_rep", [128, D], dt.float32, kind="ExternalInput")
    out = nc.dram_tensor("out", [SL, D], dt.float32, kind="ExternalOutput")

    with tile.TileContext(nc) as tc:
        import contextlib

        ctx = contextlib.ExitStack()
        with ctx:
            const = ctx.enter_context(tc.tile_pool(name="const", bufs=1))
            xt_pool = ctx.enter_context(tc.tile_pool(name="xt", bufs=2 * NDC))
            qk_pool = ctx.enter_context(tc.tile_pool(name="qk", bufs=1))
            v_pool = ctx.enter_context(tc.tile_pool(name="v", bufs=1))
            ot_pool = ctx.enter_context(tc.tile_pool(name="ot", bufs=1))
            et_pool = ctx.enter_context(tc.tile_pool(name="et", bufs=2))
            rtmp_pool = ctx.enter_context(tc.tile_pool(name="rtmp", bufs=4))
            onorm_pool = ctx.enter_context(tc.tile_pool(name="onorm", bufs=4))
            outsb_pool = ctx.enter_context(tc.tile_pool(name="outsb", bufs=2))
            mm_ps = ctx.enter_context(tc.tile_pool(name="mmps", bufs=2, space="PSUM"))
            sc_ps = ctx.enter_context(tc.tile_pool(name="scps", bufs=1, space="PSUM"))
            av_ps = ctx.enter_context(tc.tile_pool(name="avps", bufs=2, space="PSUM"))

            # ---- constants ----
            w_t = []
            for d in range(NDC):
                t = const.tile([128, 3 * D], dt.bfloat16, tag=f"wqkv{d}", name=f"wqkv{d}")
                nc.sync.dma_start(t[:], wqkvT[d * 128 : (d + 1) * 128, :])
                w_t.append(t)
            wo_t = []
            for d in range(NDC):
                t = const.tile([128, D], dt.bfloat16, tag=f"wout{d}", name=f"wout{d}")
                nc.sync.dma_start(t[:], w_outT[d * 128 : (d + 1) * 128, :])
                wo_t.append(t)
            cos_t = const.tile([128, SL], dt.bfloat16, tag="cos")
            nc.sync.dma_start(cos_t[:], cosP[:])
            sin_t = const.tile([128, SL], dt.bfloat16, tag="sin")
            nc.sync.dma_start(sin_t[:], sinP[:])
            rot_t = const.tile([128, 128], dt.bfloat16, tag="rot")
            nc.sync.dma_start(rot_t[:], rotT[:])
            ones_t = const.tile([1, 64], dt.bfloat16, tag="ones")
            nc.sync.dma_start(ones_t[:], ones64[:])
            bias_t = const.tile([128, D], dt.float32, tag="bias")
            nc.sync.dma_start(bias_t[:], bias_rep[:])

            # ---- persistent activations ----
            # q/k packed per head-pair: rows 0:64 head 2p, 64:128 head 2p+1
            qk_q = [qk_pool.tile([128, SL], dt.bfloat16, tag=f"q{p}", name=f"qkq{p}") for p in range(NPAIR)]
            qk_k = [qk_pool.tile([128, SL], dt.bfloat16, tag=f"k{p}", name=f"qkk{p}") for p in range(NPAIR)]
            # v per s-tile: [128 tokens, 12 heads x 65] (col 64 of each head = 1.0)
            v_sb = [v_pool.tile([128, NH * 65], dt.bfloat16, tag=f"v{i}", name=f"vsb{i}") for i in range(NST)]
            # transposed normalized o: d-chunk dc = heads 2dc,2dc+1
            ot_sb = [ot_pool.tile([128, SL], dt.bfloat16, tag=f"ot{d}", name=f"otsb{d}") for d in range(NDC)]

            for i in range(NST):
                vv = v_sb[i][:].rearrange("p (h c) -> p h c", h=NH)
                nc.vector.memset(vv[:, :, 64:65], 1.0)

            # ================= Phase 1: QKV projection =================
            for c in range(NSC):
                xts = []
                for d in range(NDC):
                    t = xt_pool.tile([128, SC], dt.bfloat16, tag="xt", name=f"xt_{c}_{d}")
                    nc.sync.dma_start(t[:], xT[d * 128 : (d + 1) * 128, c * SC : (c + 1) * SC])
                    xts.append(t)
                # q,k part: e-tiles 0..2*NDC-1 -> transposed output [e, s]
                for et in range(2 * NDC):
                    ps = mm_ps.tile([128, SC], dt.float32, tag="mm", name=f"qkps_{c}_{et}")
                    for d in range(NDC):
                        nc.tensor.matmul(
                            ps[:],
                            w_t[d][:, et * 128 : (et + 1) * 128],
                            xts[d][:],
                            start=(d == 0),
                            stop=(d == NDC - 1),
                        )
                    dst = qk_q[et] if et < NDC else qk_k[et - NDC]
                    nc.vector.tensor_copy(dst[:, c * SC : (c + 1) * SC], ps[:])
                # v part: natural layout via xT as stationary
                for st in range(SC // 128):
                    gst = c * (SC // 128) + st  # global s-tile
                    for nch in range(2):
                        n0, n1 = (0, 512) if nch == 0 else (512, D)
                        ps = mm_ps.tile([128, SC], dt.float32, tag="mm", name=f"vps_{gst}_{nch}")
                        for d in range(NDC):
                            nc.tensor.matmul(
                                ps[:, : n1 - n0],
                                xts[d][:, st * 128 : (st + 1) * 128],
                                w_t[d][:, 2 * D + n0 : 2 * D + n1],
                                start=(d == 0),
                                stop=(d == NDC - 1),
                            )
                        h0, h1 = (0, 8) if nch == 0 else (8, NH)
                        vv = v_sb[gst][:].rearrange("p (h c) -> p h c", h=NH)
                        pv = ps[:, : n1 - n0].rearrange("p (h c) -> p h c", c=HD)
                        nc.vector.tensor_copy(vv[:, h0:h1, 0:HD], pv[:])

            # ================= Phase 1.5: RoPE on q,k =================
            for p in range(NPAIR):
                for tens in (qk_q[p], qk_k[p]):
                    for c in range(SL // 512):
                        sl_ = slice(c * 512, (c + 1) * 512)
                        rps = mm_ps.tile([128, 512], dt.float32, tag="mm", name=f"rps_{p}_{c}")
                        nc.tensor.matmul(rps[:], rot_t[:], tens[:, sl_], start=True, stop=True)
                        t1 = rtmp_pool.tile([128, 512], dt.float32, tag="rt1", name=f"rt1_{p}_{c}")
                        nc.vector.tensor_mul(t1[:], rps[:], sin_t[:, sl_])
                        t2 = rtmp_pool.tile([128, 512], dt.float32, tag="rt2", name=f"rt2_{p}_{c}")
                        nc.vector.tensor_mul(t2[:], tens[:, sl_], cos_t[:, sl_])
                        nc.vector.tensor_add(tens[:, sl_], t1[:], t2[:])

            # ================= Phase 2: attention =================
            ActF = mybir.ActivationFunctionType
            scale = 1.0 / np.sqrt(HD)
            SPW = 2 * GK * QCS  # scores tile width: 2 heads interleaved
            for f in range(FPC):
                for p in range(NPAIR):
                    for qc in range(NQC):
                        qsl = slice(f * S + qc * QCS, f * S + (qc + 1) * QCS)
                        avp = [
                            av_ps.tile([128, QCS], dt.float32, tag="av",
                                       name=f"avp_{f}_{p}_{qc}_{hh}")
                            for hh in range(2)
                        ]
                        for g in range(NG):
                            sp = sc_ps.tile([128, SPW], dt.float32, tag="sc", name=f"sp_{f}_{p}_{qc}_{g}")
                            for j in range(GK):
                                kt = g * GK + j
                                ksl = slice(f * S + kt * 128, f * S + (kt + 1) * 128)
                                for hh in range(2):
                                    rb = 64 * hh
                                    col = (2 * j + hh) * QCS
                                    nc.tensor.matmul(
                                        sp[:, col : col + QCS],
                                        qk_k[p][rb : rb + 64, ksl],
                                        qk_q[p][rb : rb + 64, qsl],
                                        start=True,
                                        stop=True,
                                        tile_position=(rb, 0),
                                    )
                            et = et_pool.tile([128, SPW], dt.bfloat16, tag="et", name=f"et_{f}_{p}_{qc}_{g}")
                            nc.scalar.activation(et[:], sp[:], ActF.Exp, scale=float(scale))
                            for j in range(GK):
                                kt = g * GK + j
                                gst = f * KT + kt
                                for hh in range(2):
                                    h = 2 * p + hh
                                    col = (2 * j + hh) * QCS
                                    nc.tensor.matmul(
                                        avp[hh][0:65, :],
                                        v_sb[gst][:, h * 65 : h * 65 + 65],
                                        et[:, col : col + QCS],
                                        start=(kt == 0),
                                        stop=(kt == KT - 1),
                                    )
                        # normalize: o[0:64] * (1/r), r = row 64
                        for hh in range(2):
                            rb = 64 * hh
                            rc = onorm_pool.tile([1, QCS], dt.float32, tag="rc", name=f"rc_{f}_{p}_{qc}_{hh}")
                            nc.vector.reciprocal(rc[:], avp[hh][64:65, :])
                            rcb = onorm_pool.tile([1, QCS], dt.bfloat16, tag="rcb", name=f"rcb_{f}_{p}_{qc}_{hh}")
                            nc.vector.tensor_copy(rcb[:], rc[:])
                            bc = mm_ps.tile([128, QCS], dt.float32, tag="mm", name=f"bc_{f}_{p}_{qc}_{hh}")
                            nc.tensor.matmul(
                                bc[0:64, :], ones_t[:], rcb[:], start=True, stop=True
                            )
                            ou = onorm_pool.tile([64, QCS], dt.float32, tag="ou", name=f"ou_{f}_{p}_{qc}_{hh}")
                            nc.vector.tensor_copy(ou[:], avp[hh][0:64, :])
                            nc.vector.tensor_mul(
                                ot_sb[p][rb : rb + 64, qsl], ou[:], bc[0:64, :]
                            )

            # ================= Phase 3: output projection =================
            for st in range(NST):
                osb = outsb_pool.tile([128, D], dt.float32, tag="osb", name=f"osb_{st}")
                for nch in range(2):
                    n0, n1 = (0, 512) if nch == 0 else (512, D)
                    ps = mm_ps.tile([128, 512], dt.float32, tag="mm", name=f"ops_{st}_{nch}")
                    for d in range(NDC):
                        nc.tensor.matmul(
                            ps[:, : n1 - n0],
                            ot_sb[d][:, st * 128 : (st + 1) * 128],
                            wo_t[d][:, n0:n1],
                            start=(d == 0),
                            stop=(d == NDC - 1),
                        )
                    nc.vector.tensor_add(osb[:, n0:n1], ps[:, : n1 - n0], bias_t[:, n0:n1])
                nc.sync.dma_start(out[st * 128 : (st + 1) * 128, :], osb[:])

    nc.compile()
    return nc


def _prep_inputs(x, w_qkv, w_out, b_out, H, W):
    """Host-side prep: shard + transpose + cast. Returns per-core in_maps."""
    S = H * W
    SL = FPC * S
    nframes = x.shape[0] * x.shape[1]
    ncores = nframes // FPC
    xf = np.asarray(x, dtype=np.float32).reshape(nframes, S, D)

    wqkvT = np.ascontiguousarray(np.asarray(w_qkv, np.float32).T).astype(BF16)
    w_outT = np.ascontiguousarray(np.asarray(w_out, np.float32).T).astype(BF16)
    cos, sin = _host_rope(H, W, HD)  # [S, 64]
    cosP = np.tile(cos.T, (2, FPC)).astype(BF16)  # [128, SL]
    sinP = np.tile(sin.T, (2, FPC)).astype(BF16)
    rotT = _rot_matT().astype(BF16)
    ones64 = np.ones((1, 64), dtype=BF16)
    bias_rep = np.tile(np.asarray(b_out, np.float32)[None, :], (128, 1))

    in_maps = []
    for c in range(ncores):
        shard = xf[c * FPC : (c + 1) * FPC].reshape(SL, D)
        xT = np.ascontiguousarray(shard.T).astype(BF16)  # [768, SL]
        in_maps.append(
            dict(
                xT=xT,
                wqkvT=wqkvT,
                w_outT=w_outT,
                cosP=cosP,
                sinP=sinP,
                rotT=rotT,
                ones64=ones64,
                bias_rep=bias_rep,
            )
        )
    return in_maps


def run(x, w_qkv, w_out, b_out, trace=False):
    from concourse import bass_utils

    Hd, Wd = x.shape[2], x.shape[3]
    key = (Hd, Wd)
    if key not in _built:
        _built[key] = build_nc(Hd, Wd)
    nc = _built[key]
    in_maps = _prep_inputs(x, w_qkv, w_out, b_out, Hd, Wd)
    res = bass_utils.run_bass_kernel_spmd(
        nc, in_maps, core_ids=list(range(len(in_maps))), trace=trace
    )
    S = Hd * Wd
    outs = [r["out"] for r in res.results]
    full = np.concatenate(outs, axis=0).reshape(B, T, Hd, Wd, D).astype(np.float32)
    return full, res


def kernel(x, w_qkv, w_out, b_out):
    full, _ = run(x, w_qkv, w_out, b_out, trace=False)
    return full


# revision 9
# speedup vs baseline: 1.2843x; 1.2843x over previous
"""Trainium2 Bass kernel for nn_Attention_91225105367483.

Spatial attention: x[B=2,T=8,H=32,W=32,D=768] -> 16 frames x 1024 tokens.
Data-parallel over frames: 8 cores x 2 frames each. No collectives.

Per-core layout strategy (all hardcoded):
  - host pre-transposes: xT [768, 2048] (d-major), wqkvT [768, 2304],
    w_outT [768, 768], RoPE cos/sin packed [128, SL] (2 heads x 64 dims),
    rotation matrix RT [128,128] implementing rotate_half as a matmul.
  - QKV proj: q,k computed TRANSPOSED ([64 hd, seq]) so attention needs no
    on-chip transposes; v computed natural ([seq, 64]) with a ones column
    appended per head so attn@v also emits softmax denominators.
  - scores computed transposed sT=[keys, queries]; exp on ACT engine with
    scale=1/8 folded in, output bf16 straight to SBUF.
  - softmax skips max-subtraction (scores are O(1); exp is exact to 2ULP
    and fp32 accumulation makes this identical to the reference softmax).
  - o accumulated as [65, 512] in PSUM (row 64 = denominator); normalize via
    DVE reciprocal + PE broadcast matmul; out-proj from transposed o gives
    natural-layout output tiles.
"""
import sys

sys.path.insert(0, "/opt/trn_rl_repo")

import numpy as np
import ml_dtypes

BF16 = ml_dtypes.bfloat16

B, T, D = 2, 8, 768
NH, HD = 12, 64
NCORES = 8
FPC = 2  # frames per core

_built = {}


def _host_rope(H, W, head_dim):
    """Replicates reference._rope_cos_sin in numpy fp32."""
    half = head_dim // 4
    inv_freq = (1.0 / (10000.0 ** (np.arange(half, dtype=np.float32) / half))).astype(
        np.float32
    )
    th_h = np.arange(H, dtype=np.float32)[:, None] * inv_freq  # [H, half]
    th_w = np.arange(W, dtype=np.float32)[:, None] * inv_freq  # [W, half]
    cos = np.concatenate(
        [
            np.broadcast_to(np.cos(th_h)[:, None, :], (H, W, half)),
            np.broadcast_to(np.cos(th_w)[None, :, :], (H, W, half)),
        ],
        axis=-1,
    )
    sin = np.concatenate(
        [
            np.broadcast_to(np.sin(th_h)[:, None, :], (H, W, half)),
            np.broadcast_to(np.sin(th_w)[None, :, :], (H, W, half)),
        ],
        axis=-1,
    )
    cos = np.repeat(cos, 2, axis=-1).reshape(H * W, head_dim).astype(np.float32)
    sin = np.repeat(sin, 2, axis=-1).reshape(H * W, head_dim).astype(np.float32)
    return cos, sin


def _rot_matT():
    """RT = R.T where (R @ q)[2i] = -q[2i+1], (R @ q)[2i+1] = q[2i]."""
    RT = np.zeros((128, 128), dtype=np.float32)
    for i in range(64):
        RT[2 * i + 1, 2 * i] = -1.0
        RT[2 * i, 2 * i + 1] = 1.0
    return RT


def build_nc(H, W):
    """Builds the per-core Bass program. S = H*W tokens per frame."""
    import concourse.bass as bass
    import concourse.tile as tile
    from concourse import bacc, mybir

    dt = mybir.dt
    S = H * W
    SL = FPC * S  # tokens per core
    QCS = min(512, S)  # query-chunk size
    NQC = S // QCS  # query chunks per frame
    KT = S // 128  # key tiles per frame
    GK = min(2, KT)  # key tiles per exp group PER HEAD (2 heads share the tile)
    NG = KT // GK  # groups
    SC = min(512, SL)  # token chunk for qkv proj
    NSC = SL // SC
    NST = SL // 128  # s-tiles per core
    NDC = D // 128  # 6 d-chunks
    NET = 3 * D // 128  # 18 e-tiles of qkv output
    NPAIR = NH // 2  # 6 head pairs

    nc = bacc.Bacc("TRN2", target_bir_lowering=False, debug=False)

    xT = nc.dram_tensor("xT", [D, SL], dt.bfloat16, kind="ExternalInput")
    wqkvT = nc.dram_tensor("wqkvT", [D, 3 * D], dt.bfloat16, kind="ExternalInput")
    w_outT = nc.dram_tensor("w_outT", [D, D], dt.bfloat16, kind="ExternalInput")
    cosP = nc.dram_tensor("cosP", [128, SL], dt.bfloat16, kind="ExternalInput")
    sinP = nc.dram_tensor("sinP", [128, SL], dt.bfloat16, kind="ExternalInput")
    rotT = nc.dram_tensor("rotT", [128, 128], dt.bfloat16, kind="ExternalInput")
    ones64 = nc.dram_tensor("ones64", [1, 64], dt.bfloat16, kind="ExternalInput")
    bias_rep = nc.dram_tensor("bias_rep", [128, D], dt.float32, kind="ExternalInput")
    out = nc.dram_tensor("out", [SL, D], dt.float32, kind="ExternalOutput")

    with tile.TileContext(nc) as tc:
        import contextlib

        ctx = contextlib.ExitStack()
        with ctx:
            const = ctx.enter_context(tc.tile_pool(name="const", bufs=1))
            xt_pool = ctx.enter_context(tc.tile_pool(name="xt", bufs=2 * NDC))
            qk_pool = ctx.enter_context(tc.tile_pool(name="qk", bufs=1))
            v_pool = ctx.enter_context(tc.tile_pool(name="v", bufs=1))
            ot_pool = ctx.enter_context(tc.tile_pool(name="ot", bufs=1))
            et_pool = ctx.enter_context(tc.tile_pool(name="et", bufs=2))
            rtmp_pool = ctx.enter_context(tc.tile_pool(name="rtmp", bufs=4))
            onorm_pool = ctx.enter_context(tc.tile_pool(name="onorm", bufs=4))
            outsb_pool = ctx.enter_context(tc.tile_pool(name="outsb", bufs=2))
            mm_ps = ctx.enter_context(tc.tile_pool(name="mmps", bufs=2, space="PSUM"))
            sc_ps = ctx.enter_context(tc.tile_pool(name="scps", bufs=1, space="PSUM"))
            av_ps = ctx.enter_context(tc.tile_pool(name="avps", bufs=2, space="PSUM"))

            # ---- constants ----
            w_t = []
            for d in range(NDC):
                t = const.tile([128, 3 * D], dt.bfloat16, tag=f"wqkv{d}", name=f"wqkv{d}")
                nc.sync.dma_start(t[:], wqkvT[d * 128 : (d + 1) * 128, :])
                w_t.append(t)
            wo_t = []
            for d in range(NDC):
                t = const.tile([128, D], dt.bfloat16, tag=f"wout{d}", name=f"wout{d}")
                nc.sync.dma_start(t[:], w_outT[d * 128 : (d + 1) * 128, :])
                wo_t.append(t)
            cos_t = const.tile([128, SL], dt.bfloat16, tag="cos")
            nc.sync.dma_start(cos_t[:], cosP[:])
            sin_t = const.tile([128, SL], dt.bfloat16, tag="sin")
            nc.sync.dma_start(sin_t[:], sinP[:])
            rot_t = const.tile([128, 128], dt.bfloat16, tag="rot")
            nc.sync.dma_start(rot_t[:], rotT[:])
            ones_t = const.tile([1, 64], dt.bfloat16, tag="ones")
            nc.sync.dma_start(ones_t[:], ones64[:])
            bias_t = const.tile([128, D], dt.float32, tag="bias")
            nc.sync.dma_start(bias_t[:], bias_rep[:])

            # ---- persistent activations ----
            # q/k packed per head-pair: rows 0:64 head 2p, 64:128 head 2p+1
            qk_q = [qk_pool.tile([128, SL], dt.bfloat16, tag=f"q{p}", name=f"qkq{p}") for p in range(NPAIR)]
            qk_k = [qk_pool.tile([128, SL], dt.bfloat16, tag=f"k{p}", name=f"qkk{p}") for p in range(NPAIR)]
            # v per s-tile: [128 tokens, 12 heads x 65] (col 64 of each head = 1.0)
            v_sb = [v_pool.tile([128, NH * 65], dt.bfloat16, tag=f"v{i}", name=f"vsb{i}") for i in range(NST)]
            # transposed normalized o: d-chunk dc = heads 2dc,2dc+1
            ot_sb = [ot_pool.tile([128, SL], dt.bfloat16, tag=f"ot{d}", name=f"otsb{d}") for d in range(NDC)]

            for i in range(NST):
                vv = v_sb[i][:].rearrange("p (h c) -> p h c", h=NH)
                nc.vector.memset(vv[:, :, 64:65], 1.0)

            # ================= Phase 1: QKV projection =================
            for c in range(NSC):
                xts = []
                for d in range(NDC):
                    t = xt_pool.tile([128, SC], dt.bfloat16, tag="xt", name=f"xt_{c}_{d}")
                    nc.sync.dma_start(t[:], xT[d * 128 : (d + 1) * 128, c * SC : (c + 1) * SC])
                    xts.append(t)
                # q,k part: e-tiles 0..2*NDC-1 -> transposed output [e, s]
                for et in range(2 * NDC):
                    ps = mm_ps.tile([128, SC], dt.float32, tag="mm", name=f"qkps_{c}_{et}")
                    for d in range(NDC):
                        nc.tensor.matmul(
                            ps[:],
                            w_t[d][:, et * 128 : (et + 1) * 128],
                            xts[d][:],
                            start=(d == 0),
                            stop=(d == NDC - 1),
                        )
                    dst = qk_q[et] if et < NDC else qk_k[et - NDC]
                    nc.vector.tensor_copy(dst[:, c * SC : (c + 1) * SC], ps[:])
                # v part: natural layout via xT as stationary
                for st in range(SC // 128):
                    gst = c * (SC // 128) + st  # global s-tile
                    for nch in range(2):
                        n0, n1 = (0, 512) if nch == 0 else (512, D)
                        ps = mm_ps.tile([128, SC], dt.float32, tag="mm", name=f"vps_{gst}_{nch}")
                        for d in range(NDC):
                            nc.tensor.matmul(
                                ps[:, : n1 - n0],
                                xts[d][:, st * 128 : (st + 1) * 128],
                                w_t[d][:, 2 * D + n0 : 2 * D + n1],
                                start=(d == 0),
                                stop=(d == NDC - 1),
                            )
                        h0, h1 = (0, 8) if nch == 0 else (8, NH)
                        vv = v_sb[gst][:].rearrange("p (h c) -> p h c", h=NH)
                        pv = ps[:, : n1 - n0].rearrange("p (h c) -> p h c", c=HD)
                        nc.vector.tensor_copy(vv[:, h0:h1, 0:HD], pv[:])

            # ================= Phase 1.5: RoPE on q,k =================
            for p in range(NPAIR):
                for tens in (qk_q[p], qk_k[p]):
                    for c in range(SL // 512):
                        sl_ = slice(c * 512, (c + 1) * 512)
                        rps = mm_ps.tile([128, 512], dt.float32, tag="mm", name=f"rps_{p}_{c}")
                        nc.tensor.matmul(rps[:], rot_t[:], tens[:, sl_], start=True, stop=True)
                        t1 = rtmp_pool.tile([128, 512], dt.bfloat16, tag="rt1", name=f"rt1_{p}_{c}")
                        nc.vector.tensor_mul(t1[:], rps[:], sin_t[:, sl_])
                        t2 = rtmp_pool.tile([128, 512], dt.bfloat16, tag="rt2", name=f"rt2_{p}_{c}")
                        nc.vector.tensor_mul(t2[:], tens[:, sl_], cos_t[:, sl_])
                        nc.vector.tensor_add(tens[:, sl_], t1[:], t2[:])

            # ================= Phase 2: attention =================
            ActF = mybir.ActivationFunctionType
            scale = 1.0 / np.sqrt(HD)
            SPW = 2 * GK * QCS  # scores tile width: 2 heads interleaved
            for f in range(FPC):
                for p in range(NPAIR):
                    for qc in range(NQC):
                        qsl = slice(f * S + qc * QCS, f * S + (qc + 1) * QCS)
                        avp = [
                            av_ps.tile([128, QCS], dt.float32, tag="av",
                                       name=f"avp_{f}_{p}_{qc}_{hh}")
                            for hh in range(2)
                        ]
                        for g in range(NG):
                            sp = sc_ps.tile([128, SPW], dt.float32, tag="sc", name=f"sp_{f}_{p}_{qc}_{g}")
                            for j in range(GK):
                                kt = g * GK + j
                                ksl = slice(f * S + kt * 128, f * S + (kt + 1) * 128)
                                for hh in range(2):
                                    rb = 64 * hh
                                    col = (2 * j + hh) * QCS
                                    nc.tensor.matmul(
                                        sp[:, col : col + QCS],
                                        qk_k[p][rb : rb + 64, ksl],
                                        qk_q[p][rb : rb + 64, qsl],
                                        start=True,
                                        stop=True,
                                        tile_position=(rb, 0),
                                    )
                            et = et_pool.tile([128, SPW], dt.bfloat16, tag="et", name=f"et_{f}_{p}_{qc}_{g}")
                            nc.scalar.activation(et[:], sp[:], ActF.Exp, scale=float(scale))
                            for j in range(GK):
                                kt = g * GK + j
                                gst = f * KT + kt
                                for hh in range(2):
                                    h = 2 * p + hh
                                    col = (2 * j + hh) * QCS
                                    nc.tensor.matmul(
                                        avp[hh][0:65, :],
                                        v_sb[gst][:, h * 65 : h * 65 + 65],
                                        et[:, col : col + QCS],
                                        start=(kt == 0),
                                        stop=(kt == KT - 1),
                                    )
                        # normalize: o[0:64] * (1/r), r = row 64
                        # (fast approx reciprocal -> bf16 -> PE broadcast -> mult)
                        for hh in range(2):
                            rb = 64 * hh
                            rr = onorm_pool.tile([1, QCS], dt.float32, tag="rr", name=f"rr_{f}_{p}_{qc}_{hh}")
                            nc.vector.tensor_copy(rr[:], avp[hh][64:65, :])
                            rc = onorm_pool.tile([1, QCS], dt.float32, tag="rc", name=f"rc_{f}_{p}_{qc}_{hh}")
                            nc.vector.reciprocal_approx_fast(rc[:], rr[:])
                            rcb = onorm_pool.tile([1, QCS], dt.bfloat16, tag="rcb", name=f"rcb_{f}_{p}_{qc}_{hh}")
                            nc.vector.tensor_copy(rcb[:], rc[:])
                            ou = onorm_pool.tile([64, QCS], dt.float32, tag="ou", name=f"ou_{f}_{p}_{qc}_{hh}")
                            nc.vector.tensor_copy(ou[:], avp[hh][0:64, :])
                            bc = mm_ps.tile([128, QCS], dt.float32, tag="mm", name=f"bc_{f}_{p}_{qc}_{hh}")
                            nc.tensor.matmul(
                                bc[0:64, :], ones_t[:], rcb[:], start=True, stop=True
                            )
                            nc.vector.tensor_mul(
                                ot_sb[p][rb : rb + 64, qsl], ou[:], bc[0:64, :]
                            )

            # ================= Phase 3: output projection =================
            for st in range(NST):
                osb = outsb_pool.tile([128, D], dt.float32, tag="osb", name=f"osb_{st}")
                for nch in range(2):
                    n0, n1 = (0, 512) if nch == 0 else (512, D)
                    ps = mm_ps.tile([128, 512], dt.float32, tag="mm", name=f"ops_{st}_{nch}")
                    for d in range(NDC):
                        nc.tensor.matmul(
                            ps[:, : n1 - n0],
                            ot_sb[d][:, st * 128 : (st + 1) * 128],
                            wo_t[d][:, n0:n1],
                            start=(d == 0),
                            stop=(d == NDC - 1),
                        )
                    nc.vector.tensor_add(osb[:, n0:n1], ps[:, : n1 - n0], bias_t[:, n0:n1])
                nc.sync.dma_start(out[st * 128 : (st + 1) * 128, :], osb[:])

    nc.compile()
    return nc


def _prep_inputs(x, w_qkv, w_out, b_out, H, W):
    """Host-side prep: shard + transpose + cast. Returns per-core in_maps."""
    S = H * W
    SL = FPC * S
    nframes = x.shape[0] * x.shape[1]
    ncores = nframes // FPC
    xf = np.asarray(x, dtype=np.float32).reshape(nframes, S, D)

    wqkvT = np.ascontiguousarray(np.asarray(w_qkv, np.float32).T).astype(BF16)
    w_outT = np.ascontiguousarray(np.asarray(w_out, np.float32).T).astype(BF16)
    cos, sin = _host_rope(H, W, HD)  # [S, 64]
    cosP = np.tile(cos.T, (2, FPC)).astype(BF16)  # [128, SL]
    sinP = np.tile(sin.T, (2, FPC)).astype(BF16)
    rotT = _rot_matT().astype(BF16)
    ones64 = np.ones((1, 64), dtype=BF16)
    bias_rep = np.tile(np.asarray(b_out, np.float32)[None, :], (128, 1))

    in_maps = []
    for c in range(ncores):
        shard = xf[c * FPC : (c + 1) * FPC].reshape(SL, D)
        xT = np.ascontiguousarray(shard.T).astype(BF16)  # [768, SL]
        in_maps.append(
            dict(
                xT=xT,
                wqkvT=wqkvT,
                w_outT=w_outT,
                cosP=cosP,
                sinP=sinP,
                rotT=rotT,
                ones64=ones64,
                bias_rep=bias_rep,
            )
        )
    return in_maps


def run(x, w_qkv, w_out, b_out, trace=False):
    from concourse import bass_utils

    Hd, Wd = x.shape[2], x.shape[3]
    key = (Hd, Wd)
    if key not in _built:
        _built[key] = build_nc(Hd, Wd)
    nc = _built[key]
    in_maps = _prep_inputs(x, w_qkv, w_out, b_out, Hd, Wd)
    res = bass_utils.run_bass_kernel_spmd(
        nc, in_maps, core_ids=list(range(len(in_maps))), trace=trace
    )
    S = Hd * Wd
    outs = [r["out"] for r in res.results]
    full = np.concatenate(outs, axis=0).reshape(B, T, Hd, Wd, D).astype(np.float32)
    return full, res


def kernel(x, w_qkv, w_out, b_out):
    full, _ = run(x, w_qkv, w_out, b_out, trace=False)
    return full
